# revision 32
# baseline (speedup 1.0000x reference)
"""Trainium2 Bass kernel for the CAM (cross-attention module) problem.

Math (per sample b):
    img = w_img @ x1_b          # [256, 4096]
    kv  = w_txt @ x2_b          # [256, 4096]
    attn = softmax(img @ kv^T)  # [256, 256], softmax over last dim
    y = gamma * (attn @ kv) + img
    out_b = w_out @ y           # [512, 4096]

Sharding: data-parallel over batch, 16 samples -> 2 per core x 8 cores,
no collectives.

Precision: fp16 end-to-end.  x1/x2/weights are cast to fp16 on the host
(values are ~N(0,1), well inside fp16 range); every matmul runs in fp16
with exact fp32 PSUM accumulation; intermediates (img, kv, attn, y) are
stored fp16 in SBUF; the output is written fp16 to HBM and upcast on the
host.  Numpy simulation of this exact staging gives 2.5e-3 relative
error vs the fp32 reference (gate is 2e-2).  fp16 halves HBM traffic
(~23 MB/core), halves SBUF footprint, and drops PE-transposes from 1.5
to 1.0 cycles/row.

Layouts: all HBM tensors are pre-tiled on the host to exactly match
their SBUF destination layouts, so every chunk / weight / output
transfer is ONE DMA descriptor (descriptor generation on the sync queue
costs ~640ns each and was a bottleneck of the fp32 version).

The spatial contraction (attn logits) needs spatial-major operands while
the residual + output conv need channel-major ones.  img/kv are computed
channel-major, and the spatial-major copies are made with PE transposes.
The two samples per core are software-pipelined: each sample's tail
(softmax + attn@kv + output conv) is interleaved into the next sample's
projection chunks so the tensor engine never drains.
"""

import numpy as np

# Problem shapes (hardcoded per the harness contract)
B = 16
C1 = 512          # x1 channels (also output channels)
C2 = 320          # x2 channels
C2P = 384         # x2 channels padded to a multiple of 128 (K<128 matmuls
                  # and partition-offset memsets are both broken on HW)
C = 256           # projected channels
HW = 64 * 64      # spatial size
NCORES = 8
SPC = B // NCORES  # samples per core

P = 128
CH = 512           # spatial chunk width
K1 = C1 // P       # k-tiles of x1 channels
K2 = C2P // P      # k-tiles of x2 channels (padded)
KC = C // P        # k-tiles of projected channels
MO = C1 // P       # m-tiles of output conv
NCH = HW // CH     # spatial chunks
TPC = CH // P      # 128-wide spatial tiles per chunk

_BUILD_CACHE = {}


def _nonce_len():
    import inspect
    import zlib
    return 2 + (zlib.crc32(inspect.getsource(_build_nc).encode()) % 997)


def _build_nc(spc=SPC, ch=CH):
    """Build the per-core Bass program (same program on all cores)."""
    import concourse.tile as tile
    from concourse import bacc, mybir

    f32 = mybir.dt.float32
    f16 = mybir.dt.float16

    # Bacc (not plain Bass): its compile() runs move_matmul_waits_to_ldweights
    # + generate_event_semaphores, without which walrus rejects any Matmult
    # carrying more than one semaphore wait.
    nc = bacc.Bacc("TRN2", target_bir_lowering=False)
    # Host pre-tiles everything to match SBUF layouts exactly.
    x1 = nc.declare_dram_parameter("x1t", [spc, NCH, P, K1, ch], f16, isOutput=False)
    x2 = nc.declare_dram_parameter("x2t", [spc, NCH, P, K2, ch], f16, isOutput=False)
    wiT = nc.declare_dram_parameter("w_imgT", [P, K1, C], f16, isOutput=False)
    wtT = nc.declare_dram_parameter("w_txtT", [P, K2, C], f16, isOutput=False)
    woT = nc.declare_dram_parameter("w_outT", [P, KC, C1], f16, isOutput=False)
    gamma = nc.declare_dram_parameter("gamma", [1], f32, isOutput=False)
    idin = nc.declare_dram_parameter("ident", [P, P], f16, isOutput=False)
    # The PJRT executable cache fingerprints the HLO without the embedded
    # BIR payload, so two different kernels with identical I/O signatures
    # collide. A source-hash-sized dummy input makes the signature unique.
    nc.declare_dram_parameter("nonce", [1, _nonce_len()], f32, isOutput=False)
    out = nc.declare_dram_parameter("outt", [spc, NCH, P, MO, ch], f16, isOutput=True)

    Exp = mybir.ActivationFunctionType.Exp
    X = mybir.AxisListType.X

    with (
        tile.TileContext(nc) as tc,
        tc.tile_pool(name="singles", bufs=1) as singles,
        tc.tile_pool(name="xin", bufs=2) as xin,
        tc.tile_pool(name="tch", bufs=2) as tch,
        tc.tile_pool(name="imgp", bufs=2) as imgp,
        tc.tile_pool(name="kvp", bufs=NCH + 2) as kvp,
        tc.tile_pool(name="attnsb", bufs=2) as attnsb,
        tc.tile_pool(name="smalls", bufs=4) as smalls,
        tc.tile_pool(name="ostage", bufs=3) as ostage,
        tc.tile_pool(name="psA", bufs=4, space="PSUM") as psA,
        tc.tile_pool(name="psB", bufs=2, space="PSUM") as psB,
        tc.tile_pool(name="psAttn", bufs=2, space="PSUM") as psAttn,
    ):
        # ---- constants. wiT/wtT are emitted before the chunk-0 x loads
        # (small, and they unblock every matmul); woT/ident/gamma are
        # deferred past the first chunk since nothing needs them early.
        wiT_sb = singles.tile([P, K1, C], f16)
        wtT_sb = singles.tile([P, K2, C], f16)
        woT_sb = singles.tile([P, KC, C1], f16)
        ident = singles.tile([P, P], f16)
        gamma_sb = singles.tile([P, 1], f32)

        # PE p-state warmup: the tensor engine clock ramps to full speed only
        # after ~3us of continuous execution, and the first real matmul can't
        # start until its DMAs land (~11us in).  Dependency-free matmuls on a
        # memset scratch tile fill the idle window from when the PE queue
        # opens (~6.5us) and finish the ramp before real data arrives.
        warm = singles.tile([P, ch], f16)
        nc.vector.memset(warm, 0.0)
        wps = psAttn.tile([P, ch], f32, tag="attn", name="warmps")
        for _ in range(8):
            nc.tensor.matmul(wps, lhsT=warm[:, :P], rhs=warm, start=True,
                             stop=True)

        def emit_deferred_constants():
            # scalar (ACT) queue can also trigger DMAs on TRN2 — keep these
            # off the SP queue so chunk-1/2 x loads aren't descriptor-delayed
            nc.scalar.dma_start(out=ident, in_=idin[:])
            nc.scalar.dma_start(out=woT_sb, in_=woT[:])
            nc.scalar.dma_start(out=gamma_sb, in_=gamma[:].to_broadcast((P, 1)))

        # ---- per-sample emission helpers -------------------------------
        def passA_chunk(st, s, cc, first=False):
            # `first`: k-granular weight/x interleave so the first matmul's
            # inputs (wiT k0 + x1 k0, 0.3MB) land ~6us before the full-chunk
            # transfers would; later chunks use one descriptor per tensor.
            x1c = xin.tile([P, K1, ch], f16, tag="x1c", name="x1c", bufs=3)
            x2c = xin.tile([P, K2, ch], f16, tag="x2c", name="x2c")
            if first:
                nc.sync.dma_start(out=wiT_sb[:, 0, :], in_=wiT[:, 0, :])
                nc.sync.dma_start(out=x1c[:, 0, :], in_=x1[s, cc, :, 0, :])
                nc.sync.dma_start(out=wiT_sb[:, 1:, :], in_=wiT[:, 1:, :])
                nc.sync.dma_start(out=x1c[:, 1, :], in_=x1[s, cc, :, 1, :])
                nc.sync.dma_start(out=x1c[:, 2:, :], in_=x1[s, cc, :, 2:, :])
                nc.scalar.dma_start(out=wtT_sb, in_=wtT[:])
                nc.sync.dma_start(out=x2c[:, 0, :], in_=x2[s, cc, :, 0, :])
                nc.sync.dma_start(out=x2c[:, 1:, :], in_=x2[s, cc, :, 1:, :])
            else:
                nc.sync.dma_start(out=x1c, in_=x1[s, cc])
                nc.sync.dma_start(out=x2c, in_=x2[s, cc])
            for m in range(KC):
                ps = psA.tile([P, ch], f32, tag="a", name="ps_img")
                for k in range(K1):
                    nc.tensor.matmul(ps, lhsT=wiT_sb[:, k, m * P:(m + 1) * P],
                                     rhs=x1c[:, k, :],
                                     start=(k == 0), stop=(k == K1 - 1))
                nc.vector.tensor_copy(out=st["img"][:, m, cc * ch:(cc + 1) * ch],
                                      in_=ps)
            kvt = kvp.tile([P, KC, ch], f16, tag="kv", name="kvt")
            st["kvch"][cc] = kvt
            for m in range(KC):
                ps = psA.tile([P, ch], f32, tag="a", name="ps_kv")
                for k in range(K2):
                    nc.tensor.matmul(ps, lhsT=wtT_sb[:, k, m * P:(m + 1) * P],
                                     rhs=x2c[:, k, :],
                                     start=(k == 0), stop=(k == K2 - 1))
                nc.vector.tensor_copy(out=kvt[:, m, :], in_=ps)

        def transposes(st, s, pc):
            # spatial-major orientations via PE transpose of img / kv chunks.
            # Two 128-wide t-tiles share one PSUM tile so each PSUM->SBUF
            # copy moves 512 cols (halves the per-copy overhead); imgT copies
            # go to the ACT engine, txtT copies to the idle Pool engine.
            pcs = pc * ch
            imgT_c = tch.tile([P, TPC, C], f16, tag="imgT", name="imgT_c")
            for tp in range(TPC // 2):
                ps = psB.tile([P, 2, C], f16, tag="b", name="ps_imgT")
                for tt in range(2):
                    t = 2 * tp + tt
                    for i in range(KC):
                        nc.tensor.transpose(
                            ps[:, tt, i * P:(i + 1) * P],
                            st["img"][:, i, pcs + t * P:pcs + (t + 1) * P],
                            ident)
                nc.scalar.copy(out=imgT_c[:, 2 * tp:2 * tp + 2, :], in_=ps)
            txtT_c = tch.tile([P, TPC, C], f16, tag="txtT", name="txtT_c")
            for tp in range(TPC // 2):
                ps = psB.tile([P, 2, C], f16, tag="b", name="ps_txtT")
                for tt in range(2):
                    t = 2 * tp + tt
                    for i in range(KC):
                        nc.tensor.transpose(
                            ps[:, tt, i * P:(i + 1) * P],
                            st["kvch"][pc][:, i, t * P:(t + 1) * P], ident)
                nc.scalar.copy(out=txtT_c[:, 2 * tp:2 * tp + 2, :], in_=ps)
            st["imgT"][pc] = imgT_c
            st["txtT"][pc] = txtT_c

        def attn_chunk(st, s, pc):
            if st["attn_ps"] is None:
                st["attn_ps"] = [
                    psAttn.tile([P, C], f32, tag="attn", name=f"attn{s}_{m}")
                    for m in range(KC)
                ]
            for m in range(KC):
                for t in range(TPC):
                    nc.tensor.matmul(
                        st["attn_ps"][m],
                        lhsT=st["imgT"][pc][:, t, m * P:(m + 1) * P],
                        rhs=st["txtT"][pc][:, t, :],
                        start=(pc == 0 and t == 0),
                        stop=(pc == NCH - 1 and t == TPC - 1))
            st["imgT"][pc] = st["txtT"][pc] = None

        def softmax_stats(st, s):
            # softmax over the free (d) axis, gamma folded in.  DVE/ACT-only;
            # no PE instructions, so it never stalls the in-order PE queue.
            exps = []
            for m in range(KC):
                nmax = smalls.tile([P, 1], f32, tag="nmax", name="nmax")
                nc.vector.reduce_max(out=nmax, in_=st["attn_ps"][m], axis=X,
                                     negate=True)
                exp_sb = smalls.tile([P, C], f16, tag="exp", name="exp_sb")
                rsum = smalls.tile([P, 1], f32, tag="rsum", name="rsum")
                nc.scalar.activation(out=exp_sb, in_=st["attn_ps"][m], func=Exp,
                                     bias=nmax, scale=1.0, accum_out=rsum)
                rg = smalls.tile([P, 1], f32, tag="rg", name="rg")
                nc.vector.reciprocal(out=rg, in_=rsum)
                nc.vector.tensor_mul(out=rg, in0=rg, in1=gamma_sb)
                nc.vector.tensor_scalar_mul(out=exp_sb, in0=exp_sb, scalar1=rg)
                exps.append(exp_sb)
            st["exps"] = exps

        def softmax_tr(st, s):
            # transpose to attnT [d, c] for the attn@kv contraction
            attnT_sb = attnsb.tile([P, KC, C], f16, tag="attnT", name="attnT")
            st["attnT"] = attnT_sb
            for m in range(KC):
                for j in range(KC):
                    pst = psB.tile([P, P], f16, tag="b", name="ps_tr")
                    nc.tensor.transpose(pst, st["exps"][m][:, j * P:(j + 1) * P],
                                        ident)
                    nc.vector.tensor_copy(out=attnT_sb[:, j, m * P:(m + 1) * P],
                                          in_=pst)

        def softmax(st, s, cover=()):
            softmax_stats(st, s)
            for fn in cover:
                fn()
            softmax_tr(st, s)

        def ph4_chunk(st, s, cc):
            # y = gamma*attn@kv + img, overwriting img in place
            cs = cc * ch
            for m in range(KC):
                ps = psA.tile([P, ch], f32, tag="a", name="ps_ai")
                for j in range(KC):
                    nc.tensor.matmul(ps, lhsT=st["attnT"][:, j, m * P:(m + 1) * P],
                                     rhs=st["kvch"][cc][:, j, :],
                                     start=(j == 0), stop=(j == KC - 1))
                nc.vector.tensor_add(out=st["img"][:, m, cs:cs + ch], in0=ps,
                                     in1=st["img"][:, m, cs:cs + ch])
            st["kvch"][cc] = None

        def ph5_chunk(st, s, cc, last=False):
            cs = cc * ch
            ot = ostage.tile([P, MO, ch], f16, tag="ot", name="ot")
            for m2 in range(MO):
                ps = psA.tile([P, ch], f32, tag="a", name="ps_out")
                for j in range(KC):
                    nc.tensor.matmul(ps, lhsT=woT_sb[:, j, m2 * P:(m2 + 1) * P],
                                     rhs=st["img"][:, j, cs:cs + ch],
                                     start=(j == 0), stop=(j == KC - 1))
                if (m2 % 2 == 0) if last else (m2 == 0):
                    nc.vector.tensor_copy(out=ot[:, m2, :], in_=ps)
                else:
                    nc.scalar.copy(out=ot[:, m2, :], in_=ps)
                if last and m2 == MO - 3:
                    # drain the first half early on the other trigger queue
                    nc.scalar.dma_start(out=out[s, cc, :, :MO - 2, :],
                                        in_=ot[:, :MO - 2, :])
            if last:
                # final two tiles: copies ran on both engines concurrently
                nc.sync.dma_start(out=out[s, cc, :, MO - 2:, :],
                                  in_=ot[:, MO - 2:, :])
            else:
                nc.sync.dma_start(out=out[s, cc], in_=ot)

        # ---- pipelined schedule: sample s-1's tail (last transposes, attn,
        # softmax, phases 4/5) is interleaved into sample s's pass-A chunks
        # so the PE never drains at sample boundaries.
        tails = []
        for s in range(spc):
            st = {"img": None, "kvch": [None] * NCH, "attn_ps": None,
                  "attnT": None, "imgT": [None] * NCH, "txtT": [None] * NCH}
            st["img"] = imgp.tile([P, KC, HW], f16, tag="img", name=f"img{s}")
            for cc in range(NCH):
                passA_chunk(st, s, cc, first=(s == 0 and cc == 0))
                if s == 0 and cc == 1:
                    emit_deferred_constants()
                if cc >= 1:
                    transposes(st, s, cc - 1)
                if cc >= 2:
                    attn_chunk(st, s, cc - 2)
                npop = ((3, 3, 3, 3, 2, 2, 1, 1) if s < spc - 1 else
                        (4, 3, 3, 3, 2, 2, 1, 0))[min(cc, 7)]
                for _ in range(npop):
                    if tails:
                        tails.pop(0)()
            if s == spc - 1:
                # the final sample's softmax has no later pass-A to hide
                # behind; cover it with whatever of the previous sample's
                # tail is still pending (its last output-conv chunks).
                leftovers = tails[:]
                tails.clear()
                tails.extend([
                    (lambda st=st, s=s: transposes(st, s, NCH - 1)),
                    (lambda st=st, s=s: attn_chunk(st, s, NCH - 2)),
                    (lambda st=st, s=s: attn_chunk(st, s, NCH - 1)),
                    (lambda st=st, s=s, cov=tuple(leftovers):
                        softmax(st, s, cover=cov)),
                ])
            else:
                tails.extend([
                    (lambda st=st, s=s: transposes(st, s, NCH - 1)),
                    (lambda st=st, s=s: attn_chunk(st, s, NCH - 2)),
                    (lambda st=st, s=s: attn_chunk(st, s, NCH - 1)),
                    (lambda st=st, s=s: softmax_stats(st, s)),
                    (lambda st=st, s=s: softmax_tr(st, s)),
                ])
            # interleave ph4/ph5 with a one-chunk lag: outputs start DMAing
            # out ~7 chunks earlier, so the output queue isn't slammed with
            # 4MB right at the end of the sample (the DVE residual-add of
            # chunk cc gets a full chunk of cover before ph5 reads it).
            def ph45_order(s, lag=4):
                seq = [("ph4", cc) for cc in range(lag)]
                for cc in range(lag, NCH):
                    seq += [("ph5", cc - lag), ("ph4", cc)]
                seq += [("ph5", cc) for cc in range(NCH - lag, NCH)]
                return seq
            for kind, cc in ph45_order(s, lag=(4 if s < spc - 1 else 2)):
                if kind == "ph4":
                    tails.append(lambda st=st, s=s, cc=cc: ph4_chunk(st, s, cc))
                else:
                    lt = (s == spc - 1 and cc == NCH - 1)
                    tails.append(lambda st=st, s=s, cc=cc, lt=lt:
                                 ph5_chunk(st, s, cc, last=lt))
        while tails:
            tails.pop(0)()

    nc.compile()
    return nc


def _get_nc():
    key = "full"
    if key not in _BUILD_CACHE:
        _BUILD_CACHE[key] = _build_nc()
    return _BUILD_CACHE[key]


LAST_RESULTS = None  # BassKernelResults of the most recent kernel() call


def kernel(x1, x2, w_img, w_txt, w_out, gamma):
    import os
    from concourse.bass_utils import run_bass_kernel_spmd

    x1 = np.asarray(x1, dtype=np.float32).reshape(B, C1, HW)
    x2 = np.asarray(x2, dtype=np.float32).reshape(B, C2, HW)
    w_img = np.asarray(w_img, dtype=np.float32)
    w_txt = np.asarray(w_txt, dtype=np.float32)
    w_out = np.asarray(w_out, dtype=np.float32)
    gamma = np.ascontiguousarray(np.asarray(gamma, dtype=np.float32)).reshape(1)

    # pad x2 channels 320 -> 384 with zeros so every k-tile is 128 deep
    x2p = np.zeros((B, C2P, HW), dtype=np.float16)
    x2p[:, :C2, :] = x2

    # pre-tile to SBUF layouts: [sample, chunk, partition, ktile, col]
    x1t = np.ascontiguousarray(
        x1.astype(np.float16).reshape(B, K1, P, NCH, CH).transpose(0, 3, 2, 1, 4))
    x2t = np.ascontiguousarray(
        x2p.reshape(B, K2, P, NCH, CH).transpose(0, 3, 2, 1, 4))

    # weights: [partition, ktile, out-channels]
    w_imgT = np.ascontiguousarray(
        w_img.T.astype(np.float16).reshape(K1, P, C).transpose(1, 0, 2))
    w_txtT = np.zeros((C2P, C), dtype=np.float16)
    w_txtT[:C2, :] = w_txt.T
    w_txtT = np.ascontiguousarray(w_txtT.reshape(K2, P, C).transpose(1, 0, 2))
    w_outT = np.ascontiguousarray(
        w_out.T.astype(np.float16).reshape(KC, P, C1).transpose(1, 0, 2))

    nc = _get_nc()
    ident = np.eye(128, dtype=np.float16)
    in_maps = []
    for core in range(NCORES):
        s0 = core * SPC
        in_maps.append({
            "x1t": np.ascontiguousarray(x1t[s0:s0 + SPC]),
            "x2t": np.ascontiguousarray(x2t[s0:s0 + SPC]),
            "w_imgT": w_imgT,
            "w_txtT": w_txtT,
            "w_outT": w_outT,
            "gamma": gamma,
            "ident": ident,
            "nonce": np.zeros((1, _nonce_len()), dtype=np.float32),
        })

    kwargs = {}
    if os.environ.get("KERNEL_TRACE"):
        kwargs["trace"] = True
        if os.environ.get("KERNEL_TRACE_DIR"):
            kwargs["tmpdir"] = os.environ["KERNEL_TRACE_DIR"]
    res = run_bass_kernel_spmd(nc, in_maps, core_ids=list(range(NCORES)), **kwargs)
    global LAST_RESULTS
    LAST_RESULTS = res
    outs = []
    for c in range(NCORES):
        ot = res.results[c]["outt"]  # [spc, NCH, P, MO, ch] f16
        o = ot.astype(np.float32).transpose(0, 3, 2, 1, 4).reshape(SPC, C1, HW)
        outs.append(o)
    full = np.concatenate(outs, axis=0).reshape(B, C1, 64, 64)
    return full


if __name__ == "__main__":
    rng = np.random.default_rng(0)
    inputs = {
        "x1": rng.standard_normal((B, C1, 64, 64), dtype=np.float32),
        "x2": rng.standard_normal((B, C2, 64, 64), dtype=np.float32),
        "w_img": rng.standard_normal((C, C1), dtype=np.float32) / np.sqrt(C1),
        "w_txt": rng.standard_normal((C, C2), dtype=np.float32) / np.sqrt(C2),
        "w_out": rng.standard_normal((C1, C), dtype=np.float32) / np.sqrt(C),
        "gamma": rng.standard_normal(1).astype(np.float32),
    }
    out = kernel(**inputs)
    print(out.shape, out.dtype)
